# revision 3
# baseline (speedup 1.0000x reference)
"""Deep Hough Transform (histogram binning) Trainium2 Bass kernel.

out[n, c, a, r] = sum over pixels p of feat[n, c, p] * [rho_index(a, p) == r]

Strategy: per angle, the scatter-add over pixels is a dense matmul with a
one-hot matrix:  out[r, nc] = onehot[p, r].T @ featT[p, nc], accumulated over
79 pixel-chunks of 128 (K dim) in PSUM.  One-hot weights are generated
on-chip by the vector engine comparing an iota row [0..127] against the
per-pixel rho index (tensor_scalar is_equal, 4x bf16 mode), so no index
matrices ever cross HBM.

Sharding over 8 cores: 4-way over angles (25 each) x 2-way over channels
(1024 of N*C=2048 columns each).  Each core keeps its featT slice fully
resident in SBUF (~158KB/partition), so feat is read from HBM exactly once.
"""

import numpy as np
import ml_dtypes

BF16 = ml_dtypes.bfloat16

# Problem constants (hardcoded per contract; kernel.py must be self-contained)
N_B, C_B, H, W = 16, 128, 100, 100
NUM_ANGLE = 100
NUM_RHO = 100
HW = H * W                    # 10000
P = 128
NCHUNK = (HW + P - 1) // P    # 79
HWP = NCHUNK * P              # 10112
NC_TOT = N_B * C_B            # 2048
N_CORES = 8
A_SH = 4                      # angle split
NC_SH = 2                     # channel split
A_CORE = NUM_ANGLE // A_SH    # 25
NC_CORE = NC_TOT // NC_SH     # 1024
MM_N = 512                    # psum bank limit (fp32)


def _rho_index_np(H, W, numangle, numrho):
    """Same table the reference builds, pure numpy."""
    irho = float(int(np.sqrt(H * H + W * W) + 1)) / float(numrho - 1)
    itheta = np.pi / numangle
    angles = np.arange(numangle, dtype=np.float64) * itheta
    tab_cos = (np.cos(angles) / irho).astype(np.float32)
    tab_sin = (np.sin(angles) / irho).astype(np.float32)
    ys, xs = np.meshgrid(np.arange(H), np.arange(W), indexing="ij")
    xc = (xs - W // 2).astype(np.float32)
    yc = (ys - H // 2).astype(np.float32)
    r = np.round(xc[None] * tab_cos[:, None, None] + yc[None] * tab_sin[:, None, None]).astype(np.int64)
    r = r + numrho // 2
    r = np.clip(r, 0, numrho - 1)
    return r.reshape(numangle, H * W).astype(np.int32)  # [A, HW]


def _build(nchunk=NCHUNK, a_core=A_CORE, nc_core=NC_CORE, dma_groups=8, repeats=1):
    """Build the per-core Bass program (SPMD: same NEFF on all cores)."""
    import concourse.bacc as bacc
    import concourse.mybir as mybir
    import concourse.tile as tile

    f32 = mybir.dt.float32
    bf16 = mybir.dt.bfloat16
    n_mm = nc_core // MM_N
    assert nc_core % MM_N == 0

    nc = bacc.Bacc()
    featT = nc.dram_tensor("featT", [P, nchunk * nc_core], bf16, kind="ExternalInput")
    ridx = nc.dram_tensor("ridx", [P, nchunk * a_core], f32, kind="ExternalInput")
    iota = nc.dram_tensor("iota", [P, P], bf16, kind="ExternalInput")
    out = nc.dram_tensor("out", [a_core, NUM_RHO, nc_core], f32, kind="ExternalOutput")

    # chunk ranges for the feat load, one DMA each (lets PE start early)
    per = (nchunk + dma_groups - 1) // dma_groups
    ranges = [(g * per, min((g + 1) * per, nchunk)) for g in range(dma_groups)]
    ranges = [(a, b) for a, b in ranges if b > a]

    with tile.TileContext(nc) as tc:
        with (
            tc.tile_pool(name="const", bufs=1) as cpool,
            tc.tile_pool(name="onehot", bufs=4) as opool,
            tc.tile_pool(name="ycopy", bufs=2) as ypool,
            tc.tile_pool(name="psum", bufs=2, space="PSUM") as ppool,
        ):
            iota_sb = cpool.tile([P, P], bf16, tag="iota")
            nc.sync.dma_start(out=iota_sb[:], in_=iota[:])
            ridx_sb = cpool.tile([P, nchunk * a_core], f32, tag="ridx")
            nc.sync.dma_start(out=ridx_sb[:], in_=ridx[:])

            feat_tiles = []
            for gi, (c0, c1) in enumerate(ranges):
                t = cpool.tile([P, (c1 - c0) * nc_core], bf16, tag=f"feat{gi}")
                nc.sync.dma_start(
                    out=t[:], in_=featT[:, c0 * nc_core : c1 * nc_core]
                )
                feat_tiles.append((c0, c1, t))

            def feat_slice(c, j):
                for c0, c1, t in feat_tiles:
                    if c0 <= c < c1:
                        base = (c - c0) * nc_core
                        return t[:, base + j * MM_N : base + (j + 1) * MM_N]
                raise AssertionError

            for _rep in range(repeats):
                for a in range(a_core):
                    ps = [
                        ppool.tile([P, MM_N], f32, tag=f"ps{j}", name=f"ps{j}")
                        for j in range(n_mm)
                    ]
                    for c in range(nchunk):
                        oh = opool.tile([P, P], bf16, tag="oh")
                        col = c * a_core + a
                        nc.vector.tensor_scalar(
                            oh[:],
                            iota_sb[:],
                            ridx_sb[:, col : col + 1],
                            None,
                            mybir.AluOpType.is_equal,
                        )
                        for j in range(n_mm):
                            nc.tensor.matmul(
                                out=ps[j][:],
                                lhsT=oh[:],
                                rhs=feat_slice(c, j),
                                start=(c == 0),
                                stop=(c == nchunk - 1),
                            )
                    y = ypool.tile([P, nc_core], f32, tag="y")
                    for j in range(n_mm):
                        nc.scalar.copy(
                            out=y[0:NUM_RHO, j * MM_N : (j + 1) * MM_N],
                            in_=ps[j][0:NUM_RHO, :],
                        )
                    nc.sync.dma_start(out=out[a], in_=y[0:NUM_RHO, :])
    nc.finalize()
    return nc


_BUILT = {}


def _get_built(**kw):
    key = tuple(sorted(kw.items()))
    if key not in _BUILT:
        _BUILT[key] = _build(**kw)
    return _BUILT[key]


def _host_inputs(feat):
    """feat [16,128,100,100] f32 -> per-core in_maps."""
    feat = np.asarray(feat, dtype=np.float32)
    X = feat.reshape(NC_TOT, HW)

    # featT swizzled to SBUF layout [P, (chunk, nc)]
    XT = np.zeros((HWP, NC_TOT), dtype=BF16)
    XT[:HW] = X.T.astype(BF16)
    XT = XT.reshape(NCHUNK, P, NC_TOT).transpose(1, 0, 2)  # [P, NCHUNK, NC_TOT]

    ridx_full = _rho_index_np(H, W, NUM_ANGLE, NUM_RHO)  # [A, HW] int32
    RT = np.zeros((HWP, NUM_ANGLE), dtype=np.float32)
    RT[:HW] = ridx_full.T.astype(np.float32)
    RT = RT.reshape(NCHUNK, P, NUM_ANGLE).transpose(1, 0, 2)  # [P, NCHUNK, A]

    iota_arr = np.tile(np.arange(P, dtype=np.float32), (P, 1)).astype(BF16)

    in_maps = []
    for core in range(N_CORES):
        ag, ng = core // NC_SH, core % NC_SH
        fm = XT[:, :, ng * NC_CORE : (ng + 1) * NC_CORE].reshape(P, NCHUNK * NC_CORE)
        rm = RT[:, :, ag * A_CORE : (ag + 1) * A_CORE].reshape(P, NCHUNK * A_CORE)
        in_maps.append(
            {
                "featT": np.ascontiguousarray(fm),
                "ridx": np.ascontiguousarray(rm),
                "iota": iota_arr,
            }
        )
    return in_maps


def _assemble(results):
    out_full = np.empty((NUM_ANGLE, NUM_RHO, NC_TOT), dtype=np.float32)
    for core, r in enumerate(results):
        ag, ng = core // NC_SH, core % NC_SH
        out_full[
            ag * A_CORE : (ag + 1) * A_CORE, :, ng * NC_CORE : (ng + 1) * NC_CORE
        ] = r["out"]
    return np.ascontiguousarray(
        out_full.transpose(2, 0, 1).reshape(N_B, C_B, NUM_ANGLE, NUM_RHO)
    )


def kernel(feat):
    from concourse.bass_utils import run_bass_kernel_spmd

    nc = _get_built()
    in_maps = _host_inputs(feat)
    res = run_bass_kernel_spmd(nc, in_maps, core_ids=list(range(N_CORES)))
    return _assemble(res.results)
